# revision 32
# baseline (speedup 1.0000x reference)
"""Causal self-attention with sink logit on 8 Trainium2 NeuronCores.

nn_CausalSelfAttention: B=2, T=2048, C=1024, H=16, D=64.
    qkv = x @ w_qkv; per-head causal attention with a per-head sink logit in
    the softmax denominator; out = y @ w_proj.

Sharding: 8 cores = 2 batches x 4 head-groups (data-parallel over B,
tensor-parallel over heads). Each core computes its batch's qkv projection
restricted to its 4 heads, flash-style causal attention (S^T layout,
denominator via an appended ones-block in the V matmul, sink seeded into the
accumulator with a K=1 matmul), and the partial output projection against its
w_proj row-slice. The host sums the 4 per-head-group partials per batch (the
"all-reduce after c_proj", done host-side since the full output is assembled
host-side anyway).

v2 kernel: bf16 matmul inputs (f32 PSUM accumulation), software-pipelined
S->exp->AV chain (double-buffered S PSUM, S(i+1) issued before AV(i)), qkv of
group g+1 and proj of group g-1 interleaved as PE filler inside group g's
attention, shrunk diagonal score blocks (512/384/256/128 wide), and the
output projection DMA'd straight from PSUM.

kernel(**inputs) takes the FULL unsharded inputs and returns the FULL output.
"""
from contextlib import ExitStack

import numpy as np

F32 = None
BF16 = None

P_ = 128          # partitions
QB = 512          # psum bank width (fp32)
D = 64            # head dim
HPC = 4           # heads per core
NPAIR = 2
B, T, C, H = 2, 2048, 1024, 16
N_CORES = 8
GW = 512          # query group width
NG = T // GW
NTCG = GW // P_
CCH = C // P_


def _build_bass(reps=1):
    import os
    import concourse.mybir as mybir
    import concourse.tile as tile
    from concourse import bacc

    probe = os.environ.get("KERNEL_PROBE", "")

    global F32, BF16
    F32 = mybir.dt.float32
    BF16 = mybir.dt.bfloat16

    scale = 1.0 / np.sqrt(D)
    Exp = mybir.ActivationFunctionType.Exp
    MULT = mybir.AluOpType.mult

    nc = bacc.Bacc("TRN2", target_bir_lowering=False, debug=False,
                   num_devices=N_CORES)

    xt_d = nc.dram_tensor("xt", [C, T], BF16, kind="ExternalInput")
    wqk_d = nc.dram_tensor("wqk", [C, 2 * HPC * D], BF16, kind="ExternalInput")
    wv_d = nc.dram_tensor("wv", [C, HPC * D], BF16, kind="ExternalInput")
    wproj_d = nc.dram_tensor("wproj", [HPC * D, C], BF16, kind="ExternalInput")
    esones_d = nc.dram_tensor("esones", [1, HPC * P_ + QB], BF16,
                              kind="ExternalInput")
    cmask_d = nc.dram_tensor("cmask", [P_, D + 2 * P_], BF16,
                             kind="ExternalInput")
    out_d = nc.dram_tensor("out", [T, C], F32, kind="ExternalOutput")

    with tile.TileContext(nc) as tc, ExitStack() as ctx:
        pool = ctx.enter_context(tc.tile_pool(name="pool", bufs=1))
        xt_pool = ctx.enter_context(tc.tile_pool(name="xt", bufs=NG))
        work = ctx.enter_context(tc.tile_pool(name="work", bufs=2))
        psum = ctx.enter_context(tc.tile_pool(name="ps", bufs=1, space="PSUM"))

        for rep in range(reps):
            R = f"r{rep}_"
            esones = pool.tile([1, HPC * P_ + QB], BF16, tag="esones",
                               name=R + "esones")
            cmask = pool.tile([P_, D + 2 * P_], BF16, tag="cmask",
                              name=R + "cmask")
            wqk = pool.tile([P_, CCH, 2 * HPC * D], BF16, tag="wqk",
                            name=R + "wqk")
            wv = pool.tile([P_, CCH, HPC * D], BF16, tag="wv", name=R + "wv")
            wproj = pool.tile([P_, 2, C], BF16, tag="wproj", name=R + "wpr")
            QKT = pool.tile([P_, 4, T], BF16, tag="qkt", name=R + "qkt")
            VO = pool.tile([P_, T // P_, HPC, P_], BF16, tag="vo",
                           name=R + "vo")
            YT = pool.tile([P_, NPAIR, T], BF16, tag="yt", name=R + "yt")
            es = esones[0:1, 0:HPC * P_]
            ones = esones[0:1, HPC * P_:HPC * P_ + QB]
            onesc = cmask[:, 0:D]
            ident = cmask[:, D:D + P_]
            negtri = cmask[:, D + P_:D + 2 * P_]

            xqk_src = wqk_d.ap().rearrange("(c p) m -> p c m", p=P_)
            xv_src = wv_d.ap().rearrange("(c p) m -> p c m", p=P_)
            xt_src = xt_d.ap().rearrange("(c p) t -> p c t", p=P_)
            HC = CCH // 2

            xg = [xt_pool.tile([P_, CCH, GW], BF16, tag="xt",
                               name=f"{R}x{g}") for g in range(NG)]
            # interleave weight / first-group-x DMAs at fine granularity so
            # the first qk accumulation chain starts as early as possible
            QC = CCH // 4
            for q in range(4):
                nc.sync.dma_start(wqk[:, q * QC:(q + 1) * QC, :],
                                  xqk_src[:, q * QC:(q + 1) * QC, :])
                nc.scalar.dma_start(xg[0][:, q * QC:(q + 1) * QC, :],
                                    xt_src[:, q * QC:(q + 1) * QC,
                                           0 * GW:1 * GW])
            nc.sync.dma_start(esones[:], esones_d.ap())
            nc.sync.dma_start(cmask[:], cmask_d.ap())
            nc.sync.dma_start(wv[:], xv_src)
            nc.sync.dma_start(
                wproj[:], wproj_d.ap().rearrange("(co ci) m -> ci co m", ci=P_))
            for g in range(1, NG):
                for ch in range(2):
                    nc.scalar.dma_start(
                        xg[g][:, ch * HC:(ch + 1) * HC, :],
                        xt_src[:, ch * HC:(ch + 1) * HC,
                               g * GW:(g + 1) * GW])

            nc.gpsimd.tensor_copy(
                VO[:, :, :, D:P_],
                onesc[:, None, None, :].to_broadcast([P_, T // P_, HPC, D]))

            def qk_set(g, m):
                def emit():
                    tg0 = g * GW
                    ps = psum.tile([P_, GW], F32, tag="qk", bufs=2,
                                   name=f"{R}qk{g}_{m}")
                    for c in range(CCH):
                        nc.tensor.matmul(
                            ps[:], wqk[:, c, m * P_:(m + 1) * P_],
                            xg[g][:, c, :],
                            start=(c == 0), stop=(c == CCH - 1))
                    nc.vector.tensor_copy(QKT[:, m, tg0:tg0 + GW], ps[:])
                return emit, 1707.0

            def v_set(g, tcl):
                def emit():
                    ps = psum.tile([P_, HPC * D], F32, tag="qk", bufs=2,
                                   name=f"{R}vp{g}_{tcl}")
                    for c in range(CCH):
                        nc.tensor.matmul(
                            ps[:], xg[g][:, c, tcl * P_:(tcl + 1) * P_],
                            wv[:, c, :],
                            start=(c == 0), stop=(c == CCH - 1))
                    nc.vector.tensor_copy(
                        VO[:, g * NTCG + tcl, :, 0:D],
                        ps[:].rearrange("p (h d) -> p h d", h=HPC))
                return emit, 853.0

            def proj_piece(tcl, epilogue=False):
                def emit():
                    ob = work.tile([P_, C], F32, tag="ob", bufs=3,
                                   name=f"{R}ob{tcl}")
                    for nh in range(2):
                        po = psum.tile([P_, QB], F32, tag="qk", bufs=2,
                                       name=f"{R}po{tcl}_{nh}")
                        for cch in range(2):
                            nc.tensor.matmul(
                                po[:], YT[:, cch, tcl * P_:(tcl + 1) * P_],
                                wproj[:, cch, nh * QB:(nh + 1) * QB],
                                start=(cch == 0), stop=(cch == 1))
                        dst = ob[:, nh * QB:(nh + 1) * QB]
                        if epilogue and nh == 1:
                            nc.scalar.copy(dst, po[:])
                        else:
                            nc.vector.tensor_copy(dst, po[:])
                        if epilogue and "noout" not in probe:
                            nc.sync.dma_start(
                                out_d.ap()[tcl * P_:(tcl + 1) * P_,
                                           nh * QB:(nh + 1) * QB], dst)
                    if not epilogue and (
                            "noout" not in probe or tcl == 0):
                        nc.sync.dma_start(
                            out_d.ap()[tcl * P_:(tcl + 1) * P_, :], ob[:])
                return emit, 853.0

            def attn_group(g, fillers):
                tg0 = g * GW
                kdiag = g * NTCG
                total_cost = sum(cst for _, cst in fillers) or 1.0
                state = {"i": 0, "done": 0.0, "it": 0}
                n_iters = 2 * (NTCG * (g + 1) + 1)

                def pop_fill():
                    state["it"] += 1
                    target = total_cost * state["it"] / n_iters
                    while (state["i"] < len(fillers)
                           and state["done"] < target):
                        emit, cst = fillers[state["i"]]
                        emit()
                        state["i"] += 1
                        state["done"] += cst

                for p in range(NPAIR):
                    items = ([(kdiag + v, 128 * v, GW - 128 * v, True, False)
                              for v in (1, 2, 3)]
                             + [(kc, 0, GW, False, False)
                                for kc in range(kdiag)]
                             + [(kdiag, 0, GW, True, True)])
                    n = len(items)

                    def emit_S(idx, items=items, p=p):
                        kcc, off, W, diag, last = items[idx]
                        St = psum.tile([P_, 2 * GW], F32, tag="S", bufs=2,
                                       name=f"{R}S{g}_{p}_{idx}")
                        for e in range(2):
                            rows = slice(D * e, D * e + D)
                            nc.tensor.matmul(
                                St[:, e * GW:e * GW + W],
                                QKT[rows, 2 + p, kcc * P_:(kcc + 1) * P_],
                                QKT[rows, p, tg0 + off:tg0 + off + W],
                                start=True, stop=not diag)
                            if diag:
                                # causal mask: accumulate -1e5 above the
                                # diagonal of the leading 128-col block so
                                # exp() zeroes it (I.T @ negtri == negtri)
                                nc.tensor.matmul(
                                    St[:, e * GW:e * GW + P_],
                                    ident, negtri,
                                    start=False, stop=True)
                        return St

                    Sts = {0: emit_S(0)}
                    pop_fill()
                    Ys = []
                    for e in range(2):
                        h = 2 * p + e
                        Y = psum.tile([P_, GW], F32, tag="Y", bufs=2,
                                      name=f"{R}Y{g}_{p}_{e}")
                        nc.tensor.matmul(
                            Y[:], es[0:1, h * P_:(h + 1) * P_], ones[0:1, :],
                            start=True, stop=False)
                        Ys.append(Y)
                    for i in range(n):
                        kcc, off, W, diag, last = items[i]
                        St = Sts.pop(i)
                        if "noexp" in probe:
                            Pt = QKT[:, 0, 0:2 * GW]
                        else:
                            Pt = work.tile([P_, 2 * GW], BF16, tag="P",
                                           bufs=3, name=f"{R}Pt{g}_{p}_{i}")
                        if "noexp" in probe:
                            pass  # timing probe: no exp; AV reads QKT below
                        elif W == GW:
                            nc.scalar.activation(
                                Pt[:, 0:2 * GW], St[:, 0:2 * GW], Exp,
                                scale=float(scale))
                        else:
                            for e in range(2):
                                nc.scalar.activation(
                                    Pt[:, e * GW:e * GW + W],
                                    St[:, e * GW:e * GW + W], Exp,
                                    scale=float(scale))
                        if i + 1 < n:
                            Sts[i + 1] = emit_S(i + 1)
                        pop_fill()
                        if "noav" not in probe:
                            for e in range(2):
                                h = 2 * p + e
                                nc.tensor.matmul(
                                    Ys[e][:, off:off + W], VO[:, kcc, h, :],
                                    Pt[:, e * GW:e * GW + W],
                                    start=False, stop=last)
                        elif last:
                            for e in range(2):
                                nc.tensor.matmul(
                                    Ys[e][:, off:off + W], VO[:, kcc, 2 * p + e, :],
                                    Pt[:, e * GW:e * GW + W],
                                    start=False, stop=True)
                    for e in range(2):
                        scr = work.tile([P_, GW], F32, tag="scr", bufs=2,
                                        name=f"{R}sc{g}_{p}_{e}")
                        nc.vector.reciprocal(scr[D:P_, :], Ys[e][D:P_, :])
                        nc.vector.tensor_tensor(
                            YT[D * e:D * e + D, p, tg0:tg0 + GW],
                            Ys[e][0:D, :], scr[D:P_, :], MULT)
                while state["i"] < len(fillers):
                    fillers[state["i"]][0]()
                    state["i"] += 1

            for m in range(4):
                qk_set(0, m)[0]()
            for tcl in range(NTCG):
                v_set(0, tcl)[0]()
            # proj(g) becomes filler for attn(g+1); to even out group 3's
            # filler supply (it has no qkv left), half of proj(1) and all of
            # proj(2) land in attn(3).
            proj_fill = {1: [0, 1, 2, 3], 2: [4, 5],
                         3: [6, 7, 8, 9, 10, 11]}
            for g in range(NG):
                fillers = []
                if g + 1 < NG:
                    for m in range(4):
                        fillers.append(qk_set(g + 1, m))
                        fillers.append(v_set(g + 1, m))
                for tcl in proj_fill.get(g, []):
                    fillers.append(proj_piece(tcl))
                attn_group(g, fillers)
            for tcl in range((NG - 1) * NTCG, NG * NTCG):
                proj_piece(tcl, epilogue=True)[0]()

    nc.compile()
    return nc


def _make_core_inputs(x, w_qkv, w_proj, sink_logit, core):
    import ml_dtypes
    bf16 = ml_dtypes.bfloat16
    b, g = core // 4, core % 4
    h0 = g * HPC
    HD = H * D

    xt = np.ascontiguousarray(
        np.asarray(x[b], dtype=np.float32).T).astype(bf16)
    wq = w_qkv[:, h0 * D:(h0 + HPC) * D]
    wk = w_qkv[:, HD + h0 * D: HD + (h0 + HPC) * D]
    wvv = w_qkv[:, 2 * HD + h0 * D: 2 * HD + (h0 + HPC) * D]
    wqk = np.ascontiguousarray(
        np.concatenate([wq, wk], axis=1), dtype=np.float32).astype(bf16)
    wv = np.ascontiguousarray(wvv, dtype=np.float32).astype(bf16)
    wproj = np.ascontiguousarray(
        w_proj[h0 * D:(h0 + HPC) * D, :], dtype=np.float32).astype(bf16)

    esones = np.zeros((1, HPC * P_ + QB), np.float32)
    for hh in range(HPC):
        esones[0, hh * P_ + D:(hh + 1) * P_] = np.exp(
            np.asarray(sink_logit[h0 + hh], dtype=np.float64)).astype(
                np.float32)
    esones[0, HPC * P_:] = 1.0

    cmask = np.ones((P_, D + 2 * P_), np.float32)
    cmask[:, D:D + P_] = np.eye(P_, dtype=np.float32)
    # strictly-below-diagonal (key k > query j) gets a large negative logit
    cmask[:, D + P_:] = -1e5 * np.tril(np.ones((P_, P_), np.float32), -1)

    return {
        "xt": xt, "wqk": wqk, "wv": wv, "wproj": wproj,
        "esones": esones.astype(bf16),
        "cmask": cmask.astype(bf16),
    }


_CACHE = {}


def _get_runner(reps=1):
    """Build (once) the bass program and the jitted SPMD callable."""
    if reps in _CACHE:
        return _CACHE[reps]

    import jax
    from jax.experimental.shard_map import shard_map
    from jax.sharding import Mesh, NamedSharding, PartitionSpec

    import concourse.mybir as mybir
    from concourse.bass2jax import (_bass_exec_p, install_neuronx_cc_hook,
                                    partition_id_tensor)

    nc = _build_bass(reps=reps)
    install_neuronx_cc_hook()
    pid_name = nc.partition_id_tensor.name if nc.partition_id_tensor else None

    in_names, out_names, out_avals, zero_outs = [], [], [], []
    for alloc in nc.m.functions[0].allocations:
        if not isinstance(alloc, mybir.MemoryLocationSet):
            continue
        name = alloc.memorylocations[0].name
        if alloc.kind == "ExternalInput":
            if name != pid_name:
                in_names.append(name)
        elif alloc.kind == "ExternalOutput":
            out_names.append(name)
            shape = tuple(alloc.tensor_shape)
            dtype = mybir.dt.np(alloc.dtype)
            out_avals.append(jax.core.ShapedArray(shape, dtype))
            zero_outs.append(np.zeros(shape, dtype))
    n_params, n_outs = len(in_names), len(out_avals)
    all_names = in_names + out_names
    if pid_name is not None:
        all_names = all_names + [pid_name]

    def _body(*args):
        operands = list(args)
        if pid_name is not None:
            operands.append(partition_id_tensor())
        outs = _bass_exec_p.bind(
            *operands,
            out_avals=tuple(out_avals),
            in_names=tuple(all_names),
            out_names=tuple(out_names),
            lowering_input_output_aliases=(),
            sim_require_finite=True,
            sim_require_nnan=True,
            nc=nc,
        )
        return tuple(outs)

    devices = jax.devices()[:N_CORES]
    mesh = Mesh(np.asarray(devices), ("core",))
    spec = PartitionSpec("core")
    sharding = NamedSharding(mesh, spec)
    fn = jax.jit(
        shard_map(_body, mesh=mesh, in_specs=(spec,) * (n_params + n_outs),
                  out_specs=(spec,) * n_outs, check_rep=False),
        keep_unused=True)

    zeros_dev = [jax.device_put(
        np.zeros((N_CORES * z.shape[0], *z.shape[1:]), z.dtype), sharding)
        for z in zero_outs]

    meta = dict(in_names=in_names, out_names=out_names, out_avals=out_avals,
                sharding=sharding, zeros_dev=zeros_dev, jax=jax)
    _CACHE[reps] = (fn, meta)
    return fn, meta


def _device_inputs(x, w_qkv, w_proj, sink_logit, meta):
    jax = meta["jax"]
    in_maps = [_make_core_inputs(x, w_qkv, w_proj, sink_logit, core)
               for core in range(N_CORES)]
    return [
        jax.device_put(
            np.concatenate([in_maps[c][nm] for c in range(N_CORES)], axis=0),
            meta["sharding"])
        for nm in meta["in_names"]]


def kernel(x, w_qkv, w_proj, sink_logit):
    x = np.asarray(x, dtype=np.float32)
    w_qkv = np.asarray(w_qkv, dtype=np.float32)
    w_proj = np.asarray(w_proj, dtype=np.float32)
    sink_logit = np.asarray(sink_logit, dtype=np.float32)

    fn, meta = _get_runner()
    jax = meta["jax"]

    concat_in = _device_inputs(x, w_qkv, w_proj, sink_logit, meta)
    out_arrs = fn(*concat_in, *meta["zeros_dev"])
    jax.block_until_ready(out_arrs)

    i_out = meta["out_names"].index("out")
    per_core = np.asarray(out_arrs[i_out]).reshape(N_CORES, T, C)

    out = np.zeros((B, T, C), np.float64)
    for core in range(N_CORES):
        out[core // 4] += per_core[core].astype(np.float64)
    return out.astype(np.float32)
